# revision 1
# baseline (speedup 1.0000x reference)
"""NeuralTPP (GRU + monotone hazard MLP loglik) Bass kernel for 8 trn2 cores.

Problem: B=4096 samples, L=512 steps. Per step t:
  hazard:  pre = tau*w1_tau + h@w1_h.T + b1 ; a = tanh(pre)
           raw = a@w2 + b2 ; phi = softplus(raw)
           dphi = sigmoid(raw) * ((1-a^2)*w1_tau)@w2 ; lam = softplus(dphi)+eps
           tot += sum((log(lam) - phi) * m)
  GRU:     r,z,n gates with scalar input tau; h' = h + m*(1-z)*(n-h)
Output: tot / (sum(mask) + eps)   (scalar f32)

Sharding: pure data parallel, batch split 8 x 512. Each core runs the full
512-step scan on its 512 samples in H-major layout [gate-dim, batch]:
  - MM_G: one K=35 matmul/step -> PSUM rows [z_neg; r_pre; m_bcast; h_n]
          (z pre-acts negated so sigmoid gives zc = 1-z directly)
  - MM_N: K=3 matmul -> i_n (tau-only GRU n-gate input)
  - MM_P: K=35 matmul -> hazard pre for step t into row-block t%4 of a
          [128,512] PSUM bank (4 steps share a bank)
  - ACT sigmoid [64,512] -> [zc; r]; DVE [q;rh] = [zc;r] * [m_b;h_n];
    t2 = rh + i_n; ACT tanh -> n; d = n-h; f = q*d; h' = h+f
  - every 4 steps: tanh/square on the [128,512] pre bank, two K=128
    dot matmuls -> raw/s rows, copied into SBUF step-stacked tiles
  - end: batched loglik tail over [128,512] tiles (sigmoid/softplus/ln),
    per-partition sums via scalar_tensor_tensor accum_out
Host: sums the 8 cores' [128,4] partials in f64, divides by mask sum.
"""

import numpy as np

B, L, H, HH = 4096, 512, 32, 32
EPS = 1e-8
NCORES = 8
BC = B // NCORES  # 512 samples per core

_CACHE = {}


def _build_module():
    import concourse.bacc as bacc
    import concourse.mybir as mybir
    import concourse.tile as tile

    f32 = mybir.dt.float32
    AF = mybir.ActivationFunctionType
    ALU = mybir.AluOpType

    nc = bacc.Bacc()

    xr_d = nc.dram_tensor("xr", [L, 3, BC], f32, kind="ExternalInput")
    # packed consts: w1c = [lhsG | lhsN | lhsP] on 35 partitions,
    # w2c = [lhsR | lhsS | scal(c0,b2,eps)] on 128 partitions
    w1c_d = nc.dram_tensor("w1c", [35, 192], f32, kind="ExternalInput")
    w2c_d = nc.dram_tensor("w2c", [128, 11], f32, kind="ExternalInput")
    hx0_d = nc.dram_tensor("hx0", [35, BC], f32, kind="ExternalInput")
    acc_d = nc.dram_tensor("acc_out", [128, 4], f32, kind="ExternalOutput")

    with tile.TileContext(nc) as tc:
        with (
            tc.tile_pool(name="consts", bufs=1) as consts,
            tc.tile_pool(name="hx", bufs=3) as hx_pool,
            tc.tile_pool(name="work", bufs=3) as work,
            tc.tile_pool(name="grp", bufs=2) as grp,
            tc.tile_pool(name="store", bufs=1) as store,
            tc.tile_pool(name="tail", bufs=2) as tailp,
            tc.tile_pool(name="gP", bufs=2, space="PSUM") as gP,
            tc.tile_pool(name="nP", bufs=2, space="PSUM") as nP,
            tc.tile_pool(name="pP", bufs=2, space="PSUM") as pP,
            tc.tile_pool(name="dP", bufs=2, space="PSUM") as dP,
        ):
            w1c = consts.tile([35, 192], f32)
            w2c = consts.tile([128, 11], f32)
            nc.sync.dma_start(w1c[:], w1c_d[:])
            nc.sync.dma_start(w2c[:], w2c_d[:])
            lhsG, lhsN, lhsP = w1c[:, 0:128], w1c[:, 128:160], w1c[:, 160:192]
            lhsR, lhsS, c0b = w2c[:, 0:4], w2c[:, 4:8], w2c[:, 8:11]

            # raw / s values for all 512 steps, stacked 128 steps per column
            # block: value for step t lives at [t % 128, (t // 128)*512 + b]
            RAWa = store.tile([128, 4, BC], f32, tag="rawa")
            Sa = store.tile([128, 4, BC], f32, tag="sa")
            ACC = store.tile([128, 4], f32, tag="accs")

            # h carry + per-step (tau, m, 1) rows; rows: 0:32 h, 32 tau,
            # 33 m, 34 ones
            hx = hx_pool.tile([35, BC], f32, tag="hx")
            nc.sync.dma_start(hx[:], hx0_d[:])

            pbank = None
            for t in range(L):
                if t % 4 == 0:
                    pbank = pP.tile([128, BC], f32, tag="pbank")
                # hazard pre-activation for this step's h (pre-update)
                nc.tensor.matmul(
                    pbank[32 * (t % 4) : 32 * (t % 4) + 32, :],
                    lhsP,
                    hx[0:35, :],
                    start=True,
                    stop=True,
                    tile_position=(0, 32 * (t % 4)),
                )

                if t < L - 1:
                    gbank = gP.tile([128, BC], f32, tag="gbank")
                    nbank = nP.tile([32, BC], f32, tag="nbank")
                    nc.tensor.matmul(
                        gbank[:], lhsG, hx[0:35, :], start=True, stop=True
                    )
                    nc.tensor.matmul(
                        nbank[:], lhsN, hx[0:35, :], start=True, stop=True
                    )

                    S = work.tile([64, BC], f32, tag="S")
                    nc.scalar.activation(S[:], gbank[0:64, :], AF.Sigmoid)
                    U = work.tile([64, BC], f32, tag="U")
                    nc.vector.tensor_mul(U[:], S[:], gbank[64:128, :])
                    T2 = work.tile([32, BC], f32, tag="T2")
                    nc.vector.tensor_add(T2[:], U[32:64, :], nbank[:])
                    N_ = work.tile([32, BC], f32, tag="N")
                    nc.scalar.activation(N_[:], T2[:], AF.Tanh)
                    D = work.tile([32, BC], f32, tag="D")
                    nc.vector.tensor_sub(D[:], N_[:], hx[0:32, :])
                    F = work.tile([32, BC], f32, tag="F")
                    nc.vector.tensor_mul(F[:], U[0:32, :], D[:])

                    hx_next = hx_pool.tile([35, BC], f32, tag="hx")
                    nc.vector.tensor_add(hx_next[0:32, :], hx[0:32, :], F[:])
                    nc.sync.dma_start(hx_next[32:35, :], xr_d[t + 1])
                    hx = hx_next

                if t % 4 == 3:
                    g = t // 4
                    A4 = grp.tile([128, BC], f32, tag="A4")
                    nc.scalar.activation(A4[:], pbank[:], AF.Tanh)
                    SQ4 = grp.tile([128, BC], f32, tag="SQ4")
                    nc.scalar.activation(SQ4[:], A4[:], AF.Square)
                    dbank = dP.tile([64, BC], f32, tag="dbank")
                    nc.tensor.matmul(
                        dbank[0:4, :], lhsR, A4[:], start=True, stop=True,
                        tile_position=(0, 0),
                    )
                    nc.tensor.matmul(
                        dbank[32:36, :], lhsS, SQ4[:], start=True, stop=True,
                        tile_position=(0, 32),
                    )
                    blk, row = g // 32, 4 * (g % 32)
                    stR = grp.tile([4, BC], f32, tag="stR", name="stR")
                    stS = grp.tile([4, BC], f32, tag="stS", name="stS")
                    nc.scalar.activation(stR[:], dbank[0:4, :], AF.Copy)
                    nc.scalar.activation(stS[:], dbank[32:36, :], AF.Copy)
                    nc.sync.dma_start(RAWa[row : row + 4, blk, :], stR[:])
                    nc.sync.dma_start(Sa[row : row + 4, blk, :], stS[:])

            # ---- batched loglik tail ----

            Mb, SG, ND, PH, SPD, LGL, LL, LLM = ([None] * 4 for _ in range(8))
            for i in range(4):
                Mb[i] = tailp.tile([128, BC], f32, tag="Mb", name=f"Mb{i}")
                nc.sync.dma_start(Mb[i][:], xr_d[128 * i : 128 * (i + 1), 1, :])
            for i in range(4):
                SG[i] = tailp.tile([128, BC], f32, tag="SG", name=f"SG{i}")
                nc.scalar.activation(
                    SG[i][:], RAWa[:, i, :], AF.Sigmoid, bias=c0b[:, 1:2]
                )
            for i in range(4):
                ND[i] = tailp.tile([128, BC], f32, tag="ND", name=f"ND{i}")
                nc.vector.scalar_tensor_tensor(
                    ND[i][:], Sa[:, i, :], c0b[:, 0:1], SG[i][:],
                    op0=ALU.subtract, op1=ALU.mult,
                )
            # softplus(x) = ln(1 + exp(x)) — this walrus act table set has no
            # native softplus; exp and ln share natural_log_exp_and_others.
            # Ranges are small (|raw|, |dphi| < ~8) so exp cannot overflow.
            for i in range(4):
                EX = tailp.tile([128, BC], f32, tag="EX", name=f"EX{i}")
                nc.scalar.activation(EX[:], RAWa[:, i, :], AF.Exp, bias=c0b[:, 1:2])
                PH[i] = tailp.tile([128, BC], f32, tag="PH", name=f"PH{i}")
                nc.scalar.activation(PH[i][:], EX[:], AF.Ln, bias=1.0)
                EX2 = tailp.tile([128, BC], f32, tag="EX2", name=f"EX2{i}")
                nc.scalar.activation(EX2[:], ND[i][:], AF.Exp, scale=-1.0)
                SPD[i] = tailp.tile([128, BC], f32, tag="SPD", name=f"SPD{i}")
                nc.scalar.activation(SPD[i][:], EX2[:], AF.Ln, bias=1.0)
            for i in range(4):
                LGL[i] = tailp.tile([128, BC], f32, tag="LGL", name=f"LGL{i}")
                nc.scalar.activation(LGL[i][:], SPD[i][:], AF.Ln, bias=c0b[:, 2:3])
            for i in range(4):
                LL[i] = tailp.tile([128, BC], f32, tag="LL", name=f"LL{i}")
                nc.vector.tensor_sub(LL[i][:], LGL[i][:], PH[i][:])
                LLM[i] = tailp.tile([128, BC], f32, tag="LLM", name=f"LLM{i}")
                nc.vector.scalar_tensor_tensor(
                    LLM[i][:], LL[i][:], 0.0, Mb[i][:],
                    op0=ALU.add, op1=ALU.mult,
                    accum_out=ACC[:, i : i + 1],
                )
            nc.sync.dma_start(acc_d[:], ACC[:])

    nc.finalize()
    return nc


def _prep_host(inputs):
    d = {k: np.asarray(v, np.float32) for k, v in inputs.items()}
    w_ih, w_hh = d["w_ih"], d["w_hh"]
    b_ih, b_hh = d["b_ih"], d["b_hh"]
    w1, b1, w2, b2 = d["w1"], d["b1"], d["w2"], d["b2"]
    w1_tau, w1_h = w1[:, 0], w1[:, 1:]

    lhsG = np.zeros((35, 128), np.float32)
    # z_neg block (cols 0:32): gives sigmoid -> 1-z
    lhsG[0:32, 0:32] = -w_hh[32:64, :].T
    lhsG[32, 0:32] = -w_ih[32:64, 0]
    lhsG[34, 0:32] = -(b_ih[32:64] + b_hh[32:64])
    # r block
    lhsG[0:32, 32:64] = w_hh[0:32, :].T
    lhsG[32, 32:64] = w_ih[0:32, 0]
    lhsG[34, 32:64] = b_ih[0:32] + b_hh[0:32]
    # mask broadcast block
    lhsG[33, 64:96] = 1.0
    # h_n block (recurrent part of n gate, with b_hh only)
    lhsG[0:32, 96:128] = w_hh[64:96, :].T
    lhsG[34, 96:128] = b_hh[64:96]

    lhsN = np.zeros((35, 32), np.float32)
    lhsN[32, :] = w_ih[64:96, 0]
    lhsN[34, :] = b_ih[64:96]

    lhsP = np.zeros((35, 32), np.float32)
    lhsP[0:32, :] = w1_h.T
    lhsP[32, :] = w1_tau
    lhsP[34, :] = b1

    c = w1_tau * w2
    lhsR = np.zeros((128, 4), np.float32)
    lhsS = np.zeros((128, 4), np.float32)
    for g in range(4):
        lhsR[32 * g : 32 * g + 32, g] = w2
        lhsS[32 * g : 32 * g + 32, g] = c
    scal = np.tile(np.array([[c.sum(), b2[0], EPS]], np.float32), (128, 1))
    w1c = np.concatenate([lhsG, lhsN, lhsP], axis=1)  # [35, 192]
    w2c = np.concatenate([lhsR, lhsS, scal], axis=1)  # [128, 11]

    deltas, mask = d["deltas"], d["mask"]
    in_maps = []
    for i in range(NCORES):
        sl = slice(i * BC, (i + 1) * BC)
        xr = np.empty((L, 3, BC), np.float32)
        xr[:, 0, :] = deltas[sl].T
        xr[:, 1, :] = mask[sl].T
        xr[:, 2, :] = 1.0
        hx0 = np.zeros((35, BC), np.float32)
        hx0[32:35, :] = xr[0]
        in_maps.append({"xr": xr, "w1c": w1c, "w2c": w2c, "hx0": hx0})
    return in_maps


def run_on_device(inputs, trace=False):
    from concourse.bass_utils import run_bass_kernel_spmd

    if "nc" not in _CACHE:
        _CACHE["nc"] = _build_module()
    nc = _CACHE["nc"]
    in_maps = _prep_host(inputs)
    res = run_bass_kernel_spmd(nc, in_maps, core_ids=list(range(NCORES)), trace=trace)
    tot = 0.0
    for r in res.results:
        tot += np.asarray(r["acc_out"], np.float64).sum()
    msum = np.asarray(inputs["mask"], np.float64).sum()
    out = np.float32(tot / (msum + EPS))
    return np.asarray(out, np.float32), res


def kernel(**inputs):
    out, _ = run_on_device(inputs, trace=False)
    return out

